# revision 7
# baseline (speedup 1.0000x reference)
"""Multi-head self-attention (RoPE, eval-mode) Trainium2 Bass kernel.

Problem: B=2, T=2048, D=1024, H=16, d_head=64, fp32 I/O.

Sharding (8 cores): core c handles batch b=c//4 and the 4 heads
[4g, 4g+4) where g=c%4.  QKV/attention are head-local; the output
projection produces a per-core partial (contraction over this core's
256 head-dims) which the host sums across the 4 cores of each batch
and adds b_out.

Per-core layouts:
  - q,k are computed feature-major (d_head on partitions, T on free) so
    scores^T tiles come straight from matmuls; 2 heads stacked per
    128-partition tile, scores for both heads issued as row-packed
    (K=64) concurrent matmuls.
  - RoPE: rotate_half is a 32-partition block swap (done with SBUF->SBUF
    DMAs) with the sign folded into the host-provided sin table;
    q' = q*cos + rot(q)*sin_signed on DVE.
  - v is computed row-major [t, dv] and stored per head as [v | ones]
    128-wide stationary tiles, so each PV matmul yields both attn^T and
    the softmax denominators (replicated over 64 partitions) in one pass.
  - softmax skips max-subtraction (scores ~ N(0,1); exp is safe in fp32)
    and normalizes after PV with a fast reciprocal.
  - matmuls run as float32r (full-rate fp32 mode, moving dim >= 256).
"""

import numpy as np

B, T, D = 2, 2048, 1024
H = 16
DH = 64
NCORES = 8
P = 128

_CACHE = {}


def _rope_tables_np():
    theta = 1.0 / (10000.0 ** (np.arange(0, DH, 2, dtype=np.float32) / DH))
    angles = np.outer(np.arange(T, dtype=np.float32), theta)  # (T, 32)
    angles = np.concatenate([angles, angles], axis=-1)  # (T, DH)
    cos = np.cos(angles).astype(np.float32)
    sin = np.sin(angles).astype(np.float32)
    cosT = np.ascontiguousarray(cos.T)  # (64, T)
    sinT = np.ascontiguousarray(sin.T)
    sinT_signed = np.concatenate([-sinT[0:32], sinT[32:64]], axis=0)
    cos2 = np.tile(cosT, (2, 1))  # (128, T)
    sin2 = np.tile(sinT_signed, (2, 1))
    return cos2, sin2


def _build_module():
    import concourse.mybir as mybir
    import concourse.tile as tile
    from concourse import bacc

    f32 = mybir.dt.float32
    f32r = mybir.dt.float32r

    def r(ap):
        return ap.bitcast(f32r)

    nc = bacc.Bacc("TRN2", target_bir_lowering=False, debug=False)
    xT = nc.dram_tensor("xT", [D, T], f32r, kind="ExternalInput")
    w_qk = nc.dram_tensor("w_qk", [D, 512], f32r, kind="ExternalInput")
    w_v = nc.dram_tensor("w_v", [D, 256], f32r, kind="ExternalInput")
    w_o = nc.dram_tensor("w_o", [256, D], f32r, kind="ExternalInput")
    cos2 = nc.dram_tensor("cos2", [P, T], f32, kind="ExternalInput")
    sin2 = nc.dram_tensor("sin2", [P, T], f32, kind="ExternalInput")
    out = nc.dram_tensor("out", [T, D], f32, kind="ExternalOutput")

    Exp = mybir.ActivationFunctionType.Exp

    with tile.TileContext(nc) as tc:
        with tc.tile_pool(name="persist", bufs=1) as persist:
            wqk_sb = persist.tile([P, 8, 512], f32r)
            wv_sb = persist.tile([P, 8, 256], f32r)
            wo_sb = persist.tile([P, 2, 1024], f32r)
            cos_sb = persist.tile([P, T], f32)
            sin_sb = persist.tile([P, T], f32)
            q_sb = [persist.tile([P, T], f32r, tag=f"q{hp}", name=f"q{hp}") for hp in range(2)]
            k_sb = [persist.tile([P, T], f32r, tag=f"k{hp}", name=f"k{hp}") for hp in range(2)]
            # per (tk-tile, head): [v | ones] stationary 128x128
            vaug = persist.tile([P, 16, 4, P], f32r)
            attn_sb = [persist.tile([P, T], f32r, tag=f"at{hp}", name=f"at{hp}") for hp in range(2)]

            nc.sync.dma_start(wqk_sb[:], w_qk.rearrange("(dc p) c -> p dc c", p=P))
            nc.sync.dma_start(wv_sb[:], w_v.rearrange("(dc p) c -> p dc c", p=P))
            nc.sync.dma_start(wo_sb[:], w_o.rearrange("(hp p) d -> p hp d", p=P))
            nc.sync.dma_start(cos_sb[:], cos2[:])
            nc.sync.dma_start(sin_sb[:], sin2[:])
            nc.vector.memset(vaug[:, :, :, 0:64].bitcast(f32), 1.0)

            # ---- Phase B: QKV projections -------------------------------
            xT_r = xT.rearrange("(dc p) t -> p dc t", p=P)
            with (
                tc.tile_pool(name="xt", bufs=2) as xpool,
                tc.tile_pool(name="qkv_ps", bufs=4, space="PSUM") as qkps,
            ):
                for th in range(2):  # t halves of 1024
                    xt = xpool.tile([P, 8, 1024], f32r, tag="xt")
                    nc.sync.dma_start(
                        xt[:], xT_r[:, :, th * 1024 : (th + 1) * 1024]
                    )
                    for cc in range(4):  # q_hp0, q_hp1, k_hp0, k_hp1
                        for ts in range(2):
                            ps = qkps.tile([P, 512], f32, tag="fm")
                            for dc in range(8):
                                nc.tensor.matmul(
                                    ps[:],
                                    lhsT=(wqk_sb[:, dc, cc * P : (cc + 1) * P]),
                                    rhs=(xt[:, dc, ts * 512 : (ts + 1) * 512]),
                                    start=(dc == 0),
                                    stop=(dc == 7),
                                )
                            dst = (q_sb if cc < 2 else k_sb)[cc % 2]
                            o0 = th * 1024 + ts * 512
                            nc.vector.tensor_copy(dst[:, o0 : o0 + 512], ps[:])
                    for t8 in range(8):  # 128-row t-chunks for v
                        psv = qkps.tile([P, 256], f32, tag="v")
                        for dc in range(8):
                            nc.tensor.matmul(
                                psv[:],
                                lhsT=(xt[:, dc, t8 * P : (t8 + 1) * P]),
                                rhs=(wv_sb[:, dc, :]),
                                start=(dc == 0),
                                stop=(dc == 7),
                            )
                        tki = th * 8 + t8
                        nc.vector.tensor_copy(
                            vaug[:, tki, :, 64:128],
                            psv.rearrange("p (h e) -> p h e", e=64),
                        )

            # ---- Phase C: RoPE ------------------------------------------
            with tc.tile_pool(name="rope", bufs=2) as rpool:
                for hp in range(2):
                    for base in (q_sb[hp], k_sb[hp]):
                        rot = rpool.tile([P, T], f32, tag="rot")
                        for blk in range(4):
                            s = (blk ^ 1) * 32
                            nc.sync.dma_start(
                                rot[blk * 32 : (blk + 1) * 32, :],
                                base[s : s + 32, :].bitcast(f32),
                            )
                        t1 = rpool.tile([P, T], f32, tag="t1")
                        nc.vector.tensor_mul(t1[:], base[:], cos_sb[:])
                        nc.vector.tensor_mul(rot[:], rot[:], sin_sb[:])
                        nc.vector.tensor_add(base[:], t1[:], rot[:])

            # ---- Phase D: attention (flash-style over tk) ---------------
            with (
                tc.tile_pool(name="sc_ps", bufs=1, space="PSUM") as scps,
                tc.tile_pool(name="pv_ps", bufs=1, space="PSUM") as pvps,
                tc.tile_pool(name="expp", bufs=3) as epool,
                tc.tile_pool(name="norm", bufs=2) as npool,
            ):
                for hp in range(2):
                    for th in range(2):  # tq halves of 1024
                        tq0 = th * 1024
                        pv = [
                            pvps.tile([P, 1024], f32, tag=f"pv{h}", name=f"pv{h}")
                            for h in range(2)
                        ]
                        for tk in range(16):
                            sc = [
                                scps.tile([P, 1024], f32, tag=f"sc{h}", name=f"sc{h}")
                                for h in range(2)
                            ]
                            for s in range(2):
                                for h in range(2):
                                    hb = h * 64
                                    nc.tensor.matmul(
                                        sc[h][:, s * 512 : (s + 1) * 512],
                                        lhsT=(
                                            k_sb[hp][
                                                hb : hb + 64,
                                                tk * P : (tk + 1) * P,
                                            ]
                                        ),
                                        rhs=(
                                            q_sb[hp][
                                                hb : hb + 64,
                                                tq0 + s * 512 : tq0 + (s + 1) * 512,
                                            ]
                                        ),
                                        start=True,
                                        stop=True,
                                    )
                            ex = [
                                epool.tile([P, 1024], f32r, tag=f"e{h}", name=f"e{h}")
                                for h in range(2)
                            ]
                            for h in range(2):
                                nc.scalar.activation(
                                    ex[h][:], sc[h][:], Exp, scale=0.125
                                )
                            for h in range(2):
                                for s in range(2):
                                    nc.tensor.matmul(
                                        pv[h][:, s * 512 : (s + 1) * 512],
                                        lhsT=(vaug[:, tk, hp * 2 + h, :]),
                                        rhs=(ex[h][:, s * 512 : (s + 1) * 512]),
                                        start=(tk == 0),
                                        stop=(tk == 15),
                                    )
                        for h in range(2):
                            rc = npool.tile([64, 1024], f32, tag="rc")
                            nc.vector.reciprocal_approx_fast(
                                rc[:], pv[h][0:64, :]
                            )
                            hb = h * 64
                            nc.vector.tensor_mul(
                                attn_sb[hp][hb : hb + 64, tq0 : tq0 + 1024],
                                pv[h][64:128, :],
                                rc[:],
                            )

            # ---- Phase E: output projection (partial) -------------------
            with (
                tc.tile_pool(name="po_ps", bufs=4, space="PSUM") as pops,
                tc.tile_pool(name="ob", bufs=3) as opool,
            ):
                for tq in range(16):
                    for d2 in range(2):
                        po = pops.tile([P, 512], f32, tag="po")
                        for hp in range(2):
                            nc.tensor.matmul(
                                po[:],
                                lhsT=(attn_sb[hp][:, tq * P : (tq + 1) * P]),
                                rhs=(wo_sb[:, hp, d2 * 512 : (d2 + 1) * 512]),
                                start=(hp == 0),
                                stop=(hp == 1),
                            )
                        ob = opool.tile([P, 512], f32, tag="ob")
                        nc.vector.tensor_copy(ob[:], po[:])
                        nc.sync.dma_start(
                            out[tq * P : (tq + 1) * P, d2 * 512 : (d2 + 1) * 512],
                            ob[:],
                        )

    nc.compile()
    return nc


def _get_module():
    if "nc" not in _CACHE:
        _CACHE["nc"] = _build_module()
    return _CACHE["nc"]


def make_in_maps(x, w_qkv, w_out):
    cos2, sin2 = _rope_tables_np()
    in_maps = []
    for c in range(NCORES):
        b, g = divmod(c, 4)
        q0 = 256 * g
        in_maps.append(
            {
                "xT": np.ascontiguousarray(x[b].T),
                "w_qk": np.ascontiguousarray(
                    np.concatenate(
                        [
                            w_qkv[:, q0 : q0 + 256],
                            w_qkv[:, 1024 + q0 : 1024 + q0 + 256],
                        ],
                        axis=1,
                    )
                ),
                "w_v": np.ascontiguousarray(w_qkv[:, 2048 + q0 : 2048 + q0 + 256]),
                "w_o": np.ascontiguousarray(w_out[q0 : q0 + 256, :]),
                "cos2": cos2,
                "sin2": sin2,
            }
        )
    return in_maps


def combine_outputs(results, b_out):
    out = np.empty((B, T, D), dtype=np.float32)
    for b in range(B):
        acc = results[4 * b]["out"].astype(np.float32).copy()
        for c in range(4 * b + 1, 4 * b + 4):
            acc += results[c]["out"]
        out[b] = acc + b_out[None, :]
    return out


def kernel(x, w_qkv, w_out, b_out, _trace=False, _tag=[0]):
    from concourse import bass_utils

    nc = _get_module()
    in_maps = make_in_maps(
        np.asarray(x, dtype=np.float32),
        np.asarray(w_qkv, dtype=np.float32),
        np.asarray(w_out, dtype=np.float32),
    )
    res = bass_utils.run_bass_kernel_spmd(
        nc, in_maps, core_ids=list(range(NCORES)), trace=_trace
    )
    if _trace:
        _CACHE["last_result"] = res
    return combine_outputs(res.results, np.asarray(b_out, dtype=np.float32))


# revision 9
# speedup vs baseline: 1.4971x; 1.4971x over previous
"""Multi-head self-attention (RoPE, eval-mode) Trainium2 Bass kernel.

Problem: B=2, T=2048, D=1024, H=16, d_head=64, fp32 I/O.

Sharding (8 cores): core c handles batch b=c//4 and the 4 heads
[4g, 4g+4) where g=c%4.  QKV/attention are head-local; the output
projection produces a per-core partial (contraction over this core's
256 head-dims) which the host sums across the 4 cores of each batch
and adds b_out.

Per-core design notes:
  - q,k are computed feature-major (d_head on partitions, T on free) so
    scores^T tiles come straight from matmuls; 2 heads stacked per
    128-partition tile, scores for both heads issued as row-packed
    (K=64) concurrent matmuls.
  - RoPE: rotate_half is a 32-partition block swap (SBUF->SBUF DMAs)
    with the sign folded into the host-provided sin table;
    q' = q*cos + rot(q)*sin_signed, with one mul on DVE, one on GpSimd.
    Rope work is emitted interleaved with remaining QKV matmuls so the
    PE never idles long enough for HAM to re-throttle the clock.
  - v is computed row-major [t, dv] and stored per head as [ones | v]
    128-wide stationary tiles, so each PV matmul yields the softmax
    denominators (partitions 0:64, replicated) and attn^T (64:128).
  - softmax skips max-subtraction (scores ~ N(0,1), exp safe in fp32)
    and normalizes after PV with the fast DVE reciprocal (base-0 only).
  - matmuls run as float32r (single-pass fp32 mode, full rate at
    moving dim >= 256).
  - attention uses tq=512 blocks: PSUM = 2 score tiles [128,1024]
    (both heads side by side, double buffered) + 2 PV accumulators
    [128,512] = 6 banks, one exp instr per tk tile.
"""

import numpy as np

B, T, D = 2, 2048, 1024
H = 16
DH = 64
NCORES = 8
P = 128

_CACHE = {}


def _rope_tables_np():
    theta = 1.0 / (10000.0 ** (np.arange(0, DH, 2, dtype=np.float32) / DH))
    angles = np.outer(np.arange(T, dtype=np.float32), theta)  # (T, 32)
    angles = np.concatenate([angles, angles], axis=-1)  # (T, DH)
    cos = np.cos(angles).astype(np.float32)
    sin = np.sin(angles).astype(np.float32)
    cosT = np.ascontiguousarray(cos.T)  # (64, T)
    sinT = np.ascontiguousarray(sin.T)
    sinT_signed = np.concatenate([-sinT[0:32], sinT[32:64]], axis=0)
    cos2 = np.tile(cosT, (2, 1))  # (128, T)
    sin2 = np.tile(sinT_signed, (2, 1))
    return cos2, sin2


def _build_module():
    import concourse.mybir as mybir
    import concourse.tile as tile
    from concourse import bacc

    f32 = mybir.dt.float32
    f32r = mybir.dt.float32r

    nc = bacc.Bacc("TRN2", target_bir_lowering=False, debug=False)
    xT = nc.dram_tensor("xT", [D, T], f32r, kind="ExternalInput")
    w_qk = nc.dram_tensor("w_qk", [D, 512], f32r, kind="ExternalInput")
    w_v = nc.dram_tensor("w_v", [D, 256], f32r, kind="ExternalInput")
    w_o = nc.dram_tensor("w_o", [256, D], f32r, kind="ExternalInput")
    cos2 = nc.dram_tensor("cos2", [P, T], f32, kind="ExternalInput")
    sin2 = nc.dram_tensor("sin2", [P, T], f32, kind="ExternalInput")
    out = nc.dram_tensor("out", [T, D], f32, kind="ExternalOutput")

    Exp = mybir.ActivationFunctionType.Exp

    with tile.TileContext(nc) as tc:
        with tc.tile_pool(name="persist", bufs=1) as persist:
            wqk_sb = persist.tile([P, 8, 512], f32r)
            wv_sb = persist.tile([P, 8, 256], f32r)
            wo_sb = persist.tile([P, 2, 1024], f32r)
            cos_sb = persist.tile([P, T], f32)
            sin_sb = persist.tile([P, T], f32)
            # qk_sb[cc]: cc0=q_hp0, cc1=k_hp0, cc2=q_hp1, cc3=k_hp1
            qk_sb = [
                persist.tile([P, T], f32r, tag=f"qk{cc}", name=f"qk{cc}")
                for cc in range(4)
            ]
            # per (tk-tile, head): [ones | v] stationary 128x128
            vaug = persist.tile([P, 16, 4, P], f32r)
            attn_sb = [
                persist.tile([P, T], f32r, tag=f"at{hp}", name=f"at{hp}")
                for hp in range(2)
            ]

            nc.sync.dma_start(wqk_sb[:], w_qk.rearrange("(dc p) c -> p dc c", p=P))
            nc.sync.dma_start(wv_sb[:], w_v.rearrange("(dc p) c -> p dc c", p=P))
            nc.sync.dma_start(wo_sb[:], w_o.rearrange("(hp p) d -> p hp d", p=P))
            nc.sync.dma_start(cos_sb[:], cos2[:])
            nc.sync.dma_start(sin_sb[:], sin2[:])
            nc.vector.memset(vaug[:, :, :, 0:64].bitcast(f32), 1.0)

            xT_r = xT.rearrange("(dc p) t -> p dc t", p=P)

            with (
                tc.tile_pool(name="xt", bufs=2) as xpool,
                tc.tile_pool(name="qkv_ps", bufs=2, space="PSUM") as qkps,
                tc.tile_pool(name="rope", bufs=2) as rpool,
            ):

                def fm_chain(xt, tq, cc):
                    """One feature-major QKV chain -> qk_sb[cc] t-slice."""
                    ps = qkps.tile([P, 512], f32, tag="fm", name="fmps")
                    for dc in range(8):
                        nc.tensor.matmul(
                            ps[:],
                            lhsT=wqk_sb[:, dc, cc * P : (cc + 1) * P],
                            rhs=xt[:, dc, :],
                            start=(dc == 0),
                            stop=(dc == 7),
                        )
                    o0 = tq * 512
                    nc.vector.tensor_copy(qk_sb[cc][:, o0 : o0 + 512], ps[:])

                def v_chain(xt, tq, t4):
                    psv = qkps.tile([P, 256], f32, tag="v", name="vps")
                    for dc in range(8):
                        nc.tensor.matmul(
                            psv[:],
                            lhsT=xt[:, dc, t4 * P : (t4 + 1) * P],
                            rhs=wv_sb[:, dc, :],
                            start=(dc == 0),
                            stop=(dc == 7),
                        )
                    tki = tq * 4 + t4
                    nc.vector.tensor_copy(
                        vaug[:, tki, :, 64:128],
                        psv.rearrange("p (h e) -> p h e", e=64),
                    )

                def rope(hp):
                    """Apply RoPE in place to q and k of head-pair hp."""
                    for base in (qk_sb[2 * hp], qk_sb[2 * hp + 1]):
                        rot = rpool.tile([P, T], f32, tag="rot", name="rot")
                        for blk in range(4):
                            s = (blk ^ 1) * 32
                            nc.sync.dma_start(
                                rot[blk * 32 : (blk + 1) * 32, :],
                                base[s : s + 32, :].bitcast(f32),
                            )
                        t1 = rpool.tile([P, T], f32, tag="t1", name="t1")
                        nc.vector.tensor_mul(t1[:], base[:], cos_sb[:])
                        nc.gpsimd.tensor_mul(rot[:], rot[:], sin_sb[:])
                        nc.vector.tensor_add(base[:], t1[:], rot[:])

                xts = []
                for tq in range(4):
                    xt = xpool.tile([P, 8, 512], f32r, tag="xt", name="xt")
                    nc.sync.dma_start(xt[:], xT_r[:, :, tq * 512 : (tq + 1) * 512])
                    xts.append(xt)

                # Quarters 0-2: everything.  Quarter 3: hp0's q,k first so
                # rope(0) can run under the remaining PE work; rope(1) is
                # emitted last and overlaps the start of attention-hp0.
                for tq in range(3):
                    for cc in range(4):
                        fm_chain(xts[tq], tq, cc)
                    for t4 in range(4):
                        v_chain(xts[tq], tq, t4)
                fm_chain(xts[3], 3, 0)
                fm_chain(xts[3], 3, 1)
                rope(0)
                fm_chain(xts[3], 3, 2)
                fm_chain(xts[3], 3, 3)
                for t4 in range(4):
                    v_chain(xts[3], 3, t4)
                rope(1)

            # ---- attention (flash-style over tk) ------------------------
            with (
                tc.tile_pool(name="sc_ps", bufs=2, space="PSUM") as scps,
                tc.tile_pool(name="pv_ps", bufs=1, space="PSUM") as pvps,
                tc.tile_pool(name="expp", bufs=4) as epool,
                tc.tile_pool(name="norm", bufs=2) as npool,
            ):
                for hp in range(2):
                    for tq in range(4):  # tq blocks of 512
                        tq0 = tq * 512
                        pv = [
                            pvps.tile([P, 512], f32, tag=f"pv{h}", name=f"pv{h}")
                            for h in range(2)
                        ]
                        for tk in range(16):
                            sc = scps.tile([P, 1024], f32, tag="sc", name="sc")
                            for h in range(2):
                                hb = h * 64
                                nc.tensor.matmul(
                                    sc[:, h * 512 : (h + 1) * 512],
                                    lhsT=qk_sb[2 * hp + 1][
                                        hb : hb + 64, tk * P : (tk + 1) * P
                                    ],
                                    rhs=qk_sb[2 * hp][hb : hb + 64, tq0 : tq0 + 512],
                                    start=True,
                                    stop=True,
                                )
                            ex = epool.tile([P, 1024], f32r, tag="e", name="e")
                            nc.scalar.activation(ex[:], sc[:], Exp, scale=0.125)
                            for h in range(2):
                                nc.tensor.matmul(
                                    pv[h][:],
                                    lhsT=vaug[:, tk, hp * 2 + h, :],
                                    rhs=ex[:, h * 512 : (h + 1) * 512],
                                    start=(tk == 0),
                                    stop=(tk == 15),
                                )
                        for h in range(2):
                            rc = npool.tile([64, 512], f32, tag="rc", name="rc")
                            nc.vector.reciprocal_approx_fast(rc[:], pv[h][0:64, :])
                            hb = h * 64
                            nc.vector.tensor_mul(
                                attn_sb[hp][hb : hb + 64, tq0 : tq0 + 512],
                                pv[h][64:128, :],
                                rc[:],
                            )

            # ---- output projection (partial) ----------------------------
            with (
                tc.tile_pool(name="po_ps", bufs=4, space="PSUM") as pops,
                tc.tile_pool(name="ob", bufs=3) as opool,
            ):
                for tq in range(16):
                    for d2 in range(2):
                        po = pops.tile([P, 512], f32, tag="po", name="po")
                        for hp in range(2):
                            nc.tensor.matmul(
                                po[:],
                                lhsT=attn_sb[hp][:, tq * P : (tq + 1) * P],
                                rhs=wo_sb[:, hp, d2 * 512 : (d2 + 1) * 512],
                                start=(hp == 0),
                                stop=(hp == 1),
                            )
                        ob = opool.tile([P, 512], f32, tag="ob", name="ob")
                        nc.vector.tensor_copy(ob[:], po[:])
                        nc.sync.dma_start(
                            out[tq * P : (tq + 1) * P, d2 * 512 : (d2 + 1) * 512],
                            ob[:],
                        )

    nc.compile()
    return nc


def _get_module():
    if "nc" not in _CACHE:
        _CACHE["nc"] = _build_module()
    return _CACHE["nc"]


def make_in_maps(x, w_qkv, w_out):
    cos2, sin2 = _rope_tables_np()
    in_maps = []
    for c in range(NCORES):
        b, g = divmod(c, 4)
        q0 = 256 * g
        # column chunks: [q_hp0 | k_hp0 | q_hp1 | k_hp1]
        wqk_c = np.concatenate(
            [
                w_qkv[:, q0 : q0 + 128],
                w_qkv[:, 1024 + q0 : 1024 + q0 + 128],
                w_qkv[:, q0 + 128 : q0 + 256],
                w_qkv[:, 1024 + q0 + 128 : 1024 + q0 + 256],
            ],
            axis=1,
        )
        in_maps.append(
            {
                "xT": np.ascontiguousarray(x[b].T),
                "w_qk": np.ascontiguousarray(wqk_c),
                "w_v": np.ascontiguousarray(w_qkv[:, 2048 + q0 : 2048 + q0 + 256]),
                "w_o": np.ascontiguousarray(w_out[q0 : q0 + 256, :]),
                "cos2": cos2,
                "sin2": sin2,
            }
        )
    return in_maps


def combine_outputs(results, b_out):
    out = np.empty((B, T, D), dtype=np.float32)
    for b in range(B):
        acc = results[4 * b]["out"].astype(np.float32).copy()
        for c in range(4 * b + 1, 4 * b + 4):
            acc += results[c]["out"]
        out[b] = acc + b_out[None, :]
    return out


def kernel(x, w_qkv, w_out, b_out, _trace=False, _tag=[0]):
    from concourse import bass_utils

    nc = _get_module()
    in_maps = make_in_maps(
        np.asarray(x, dtype=np.float32),
        np.asarray(w_qkv, dtype=np.float32),
        np.asarray(w_out, dtype=np.float32),
    )
    res = bass_utils.run_bass_kernel_spmd(
        nc, in_maps, core_ids=list(range(NCORES)), trace=_trace
    )
    if _trace:
        _CACHE["last_result"] = res
    return combine_outputs(res.results, np.asarray(b_out, dtype=np.float32))
